# revision 6
# baseline (speedup 1.0000x reference)
"""Contrastive loss kernel for Trainium2 (8 NeuronCores, SPMD).

Math: loss = mean_{pos pairs}(1-cos_sim)^2 + mean_{neg pairs}relu(cos_sim-1)^2
with pos = same-label upper-triangle pairs, neg = different-label ordered pairs.

Strategy:
  * Host sorts rows by label so same-label pairs form a narrow diagonal band,
    and rotates columns per core so the band lands at the same local columns
    on every core (one uniform SPMD program).
  * Each core computes its [512, 4096] slice of the Gram matrix in bf16 on
    the PE (raw, unnormalized rhs; lhsT pre-scaled by 1/norm).
  * Norms come from a row-major squared-sum pipeline (ScalarE activation
    accumulate), inverted on VectorE in a compact [128, 32] layout, and
    broadcast along partitions via a K=1 ones-matmul.
  * Epilogue per PSUM tile: multiply by inv_j (column side of normalization),
    relu(s-1) then Square-accumulate => neg partials over ALL pairs; on the
    diagonal band only, index masks (computed from targets-derived per-row
    bounds) give the pos partials and a same-label correction to subtract
    from the neg sum.
  * Exact pair counts are integer combinatorics of targets, done on host.
    Host combines 8 x [128, 16] partial-stat tensors into the final scalar.
"""

import numpy as np
import ml_dtypes

import concourse.bass as bass
import concourse.bacc as bacc
import concourse.mybir as mybir
import concourse.tile as tile

N, D, NCORES = 4096, 512, 8
RPC = N // NCORES  # 512 rows per core
BAND_W = 512       # band slice width (covers all same-label cols per strip)
BMAX = 192         # max same-label block size the fixed band supports

F32 = mybir.dt.float32
BF16 = mybir.dt.bfloat16
AF = mybir.ActivationFunctionType
ALU = mybir.AluOpType


def build_program():
    nc = bacc.Bacc(None)
    xt16_d = nc.declare_dram_parameter("xt16", [D, N], BF16, isOutput=False)
    xr16_d = nc.declare_dram_parameter("xr16", [N, D], BF16, isOutput=False)
    meta_d = nc.declare_dram_parameter("meta", [128, BAND_W + 16], F32,
                                       isOutput=False)
    stats_d = nc.declare_dram_parameter("stats", [128, 16], F32, isOutput=True)
    scratch = nc.dram_tensor("invbounce", [N], F32)

    with tile.TileContext(nc) as tc:
        with (
            tc.tile_pool(name="perm", bufs=1) as perm,
            tc.tile_pool(name="rows", bufs=32) as rows,
            tc.tile_pool(name="rjunk", bufs=2) as rjunk,
            tc.tile_pool(name="work", bufs=2) as work,
            tc.tile_pool(name="bandp", bufs=2) as bandp,
            tc.tile_pool(name="psum", bufs=2, space="PSUM") as psum,
        ):
            meta_t = perm.tile([128, BAND_W + 16], F32, tag="meta")
            nc.sync.dma_start(meta_t[:], meta_d[:])
            iota_t = meta_t[:, 0:BAND_W]
            aux_t = meta_t[:, BAND_W:BAND_W + 16]
            stats_t = perm.tile([128, 16], F32, tag="stats")
            sumsq = perm.tile([128, 32], F32, tag="sumsq")
            xt_c = [perm.tile([128, N], BF16, tag=f"xt{k}", name=f"xt{k}") for k in range(4)]
            invf = perm.tile([128, N], F32, tag="invf")
            inv16own = perm.tile([128, RPC], BF16, tag="inv16own")
            xtL = [perm.tile([128, RPC], BF16, tag=f"xtL{k}", name=f"xtL{k}") for k in range(4)]
            ones1 = perm.tile([1, 128], F32, tag="ones1")
            nc.vector.memset(ones1[:], 1.0)
            flatF = perm.tile([1, N], F32, tag="flatF")
            nrm = perm.tile([128, 32], F32, tag="nrm")
            nrmx = perm.tile([128, 32], F32, tag="nrmx")
            invr = perm.tile([128, 32], F32, tag="invr")

            # --- row-major norms pipeline (overlaps DMA) ---
            for t in range(32):
                rt = rows.tile([128, D], BF16, tag="rt")
                nc.sync.dma_start(rt[:], xr16_d[128 * t:128 * (t + 1), :])
                jk = rjunk.tile([128, D], BF16, tag="rj")
                nc.scalar.activation(jk[:], rt[:], AF.Square,
                                     accum_out=sumsq[:, t:t + 1])

            # --- transposed-chunk DMAs, first halves first ---
            for h in range(2):
                for k in range(4):
                    nc.sync.dma_start(
                        xt_c[k][:, 2048 * h:2048 * (h + 1)],
                        xt16_d[128 * k:128 * (k + 1), 2048 * h:2048 * (h + 1)])

            # --- inv = 1/max(sqrt(sumsq), eps), in compact layout ---
            nc.scalar.activation(nrm[:], sumsq[:], AF.Sqrt)
            nc.vector.tensor_scalar(out=nrmx[:], in0=nrm[:], scalar1=1e-8,
                                    scalar2=None, op0=ALU.max)
            nc.vector.reciprocal(invr[:], nrmx[:])

            # --- bounce through DRAM to get [1, 4096] column-ordered inv ---
            nc.sync.dma_start(scratch[:].rearrange("(p t) -> p t", p=128),
                              invr[:])
            nc.sync.dma_start(flatF[0:1, :],
                              scratch[:].rearrange("(o f) -> o f", o=1))

            # --- broadcast inv along partitions via K=1 ones matmul ---
            for h in range(2):
                mg = psum.tile([128, 2048], F32, tag="mega")
                for t4 in range(4):
                    nc.tensor.matmul(
                        mg[:, 512 * t4:512 * (t4 + 1)],
                        ones1[0:1, :],
                        flatF[0:1, 2048 * h + 512 * t4:2048 * h + 512 * (t4 + 1)],
                        start=True, stop=True)
                nc.scalar.activation(invf[:, 2048 * h:2048 * (h + 1)], mg[:],
                                     AF.Copy)

            # --- lhsT = own columns scaled by inv (bf16) ---
            nc.scalar.activation(inv16own[:], invf[:, 256:768], AF.Copy)
            for k in range(4):
                nc.vector.tensor_tensor(xtL[k][:], xt_c[k][:, 256:768],
                                        inv16own[:], ALU.mult)

            # --- Gram megatiles + epilogue ---
            for h in range(2):
                for s in range(4):
                    mi = 4 * h + s
                    sim = psum.tile([128, 2048], F32, tag="mega")
                    for t4 in range(4):
                        for k in range(4):
                            nc.tensor.matmul(
                                sim[:, 512 * t4:512 * (t4 + 1)],
                                xtL[k][:, 128 * s:128 * (s + 1)],
                                xt_c[k][:, 2048 * h + 512 * t4:
                                          2048 * h + 512 * (t4 + 1)],
                                start=(k == 0), stop=(k == 3))
                    sb = work.tile([128, 2048], BF16, tag="sb")
                    nc.vector.tensor_tensor(sb[:], sim[:],
                                            invf[:, 2048 * h:2048 * (h + 1)],
                                            ALU.mult)
                    rb = work.tile([128, 2048], BF16, tag="rb")
                    nc.vector.tensor_scalar(out=rb[:], in0=sb[:], scalar1=1.0,
                                            scalar2=0.0, op0=ALU.subtract,
                                            op1=ALU.max)
                    jk2 = work.tile([128, 2048], BF16, tag="jk")
                    nc.scalar.activation(jk2[:], rb[:], AF.Square,
                                         accum_out=stats_t[:, mi:mi + 1])
                    if h == 0:
                        a = 64 + 128 * s
                        u1 = bandp.tile([128, BAND_W], BF16, tag="u1")
                        nc.vector.tensor_scalar(out=u1[:], in0=sb[:, a:a + BAND_W],
                                                scalar1=1.0, scalar2=None,
                                                op0=ALU.subtract)
                        chi = bandp.tile([128, BAND_W], BF16, tag="chi")
                        nc.vector.tensor_scalar(out=chi[:], in0=iota_t,
                                                scalar1=aux_t[:, 4 * s + 2:4 * s + 3],
                                                scalar2=None, op0=ALU.is_lt)
                        b1 = bandp.tile([128, BAND_W], BF16, tag="b1")
                        nc.vector.tensor_scalar(out=b1[:], in0=iota_t,
                                                scalar1=aux_t[:, 4 * s:4 * s + 1],
                                                scalar2=None, op0=ALU.is_gt)
                        a1 = bandp.tile([128, BAND_W], BF16, tag="a1")
                        nc.vector.tensor_scalar(out=a1[:], in0=iota_t,
                                                scalar1=aux_t[:, 4 * s + 1:4 * s + 2],
                                                scalar2=None, op0=ALU.is_ge)
                        pu = bandp.tile([128, BAND_W], BF16, tag="pu")
                        nc.vector.tensor_tensor(pu[:], b1[:], chi[:], ALU.mult)
                        tm = bandp.tile([128, BAND_W], BF16, tag="tm")
                        nc.vector.tensor_tensor(tm[:], a1[:], chi[:], ALU.mult)
                        v = bandp.tile([128, BAND_W], BF16, tag="v")
                        nc.vector.tensor_tensor(v[:], u1[:], pu[:], ALU.mult)
                        g = bandp.tile([128, BAND_W], BF16, tag="g")
                        nc.vector.tensor_tensor(g[:], rb[:, a:a + BAND_W],
                                                tm[:], ALU.mult)
                        bj1 = bandp.tile([128, BAND_W], BF16, tag="bj1")
                        nc.scalar.activation(bj1[:], v[:], AF.Square,
                                             accum_out=stats_t[:, 8 + s:9 + s])
                        bj2 = bandp.tile([128, BAND_W], BF16, tag="bj2")
                        nc.scalar.activation(bj2[:], g[:], AF.Square,
                                             accum_out=stats_t[:, 12 + s:13 + s])

            nc.sync.dma_start(stats_d[:], stats_t[:])
    nc.finalize()
    return nc


def host_prepare(inputs, targets):
    """Sort/rotate/pack per-core inputs. Returns (in_maps, counts)."""
    inputs = np.asarray(inputs, np.float32)
    targets_np = np.asarray(targets)
    order = np.argsort(targets_np, kind="stable")
    tss = targets_np[order]
    X = inputs[order]
    lo = np.searchsorted(tss, tss, side="left").astype(np.int64)
    hi = np.searchsorted(tss, tss, side="right").astype(np.int64)
    bmax = int((hi - lo).max())
    if bmax > BMAX:
        raise NotImplementedError(
            f"label block of size {bmax} exceeds supported band ({BMAX})")

    X16 = X.astype(ml_dtypes.bfloat16)
    # device row r holds local column j(r) = 32*(r%128) + r//128 so the
    # [128, 32] inv tile flattens linearly through the DRAM bounce
    r = np.arange(N)
    j_of_r = 32 * (r % 128) + r // 128


    in_maps = []
    for c in range(NCORES):
        off = (RPC * c - 256) % N
        colmap = (np.arange(N) + off) % N  # local j -> global sorted row
        Xc = X16[colmap, :]
        xt16_c = np.ascontiguousarray(Xc.T)
        xr16_c = np.ascontiguousarray(Xc[j_of_r, :])
        meta = np.zeros((128, BAND_W + 16), np.float32)
        meta[:, 0:BAND_W] = np.arange(BAND_W, dtype=np.float32)[None, :]
        aux = meta[:, BAND_W:BAND_W + 16]
        for s in range(4):
            a_s = 64 + 128 * s
            gi = RPC * c + 128 * s + np.arange(128)
            base = RPC * c - 256 + a_s
            i_cmp = (gi - base).astype(np.float32)
            lo_cmp = (lo[gi] - base).astype(np.float32)
            hi_cmp = (hi[gi] - base).astype(np.float32)
            assert (lo_cmp >= 0).all() and (hi_cmp <= BAND_W).all()
            aux[:, 4 * s + 0] = i_cmp
            aux[:, 4 * s + 1] = lo_cmp
            aux[:, 4 * s + 2] = hi_cmp
        in_maps.append({
            "xt16": xt16_c,
            "xr16": xr16_c,
            "meta": meta,
        })

    cnts = np.bincount(targets_np.astype(np.int64))
    pos_cnt = float((cnts * (cnts - 1) // 2).sum())
    neg_cnt = float(N * N - (cnts * cnts).sum())
    return in_maps, pos_cnt, neg_cnt


def combine(stats_list, pos_cnt, neg_cnt):
    neg_all = 0.0
    pos_sum = 0.0
    corr = 0.0
    for st in stats_list:
        st = np.asarray(st, np.float64)
        neg_all += st[:, 0:8].sum()
        pos_sum += st[:, 8:12].sum()
        corr += st[:, 12:16].sum()
    loss = np.float32(pos_sum / pos_cnt + (neg_all - corr) / neg_cnt)
    return np.asarray(loss, np.float32)


_prog_cache = {}


def kernel(inputs, targets):
    from concourse.bass_utils import run_bass_kernel_spmd
    in_maps, pos_cnt, neg_cnt = host_prepare(inputs, targets)
    if "nc" not in _prog_cache:
        _prog_cache["nc"] = build_program()
    nc = _prog_cache["nc"]
    res = run_bass_kernel_spmd(nc, in_maps, list(range(NCORES)))
    stats_list = [res.results[c]["stats"] for c in range(NCORES)]
    return combine(stats_list, pos_cnt, neg_cnt)


# revision 7
# speedup vs baseline: 1.0728x; 1.0728x over previous
"""Contrastive loss kernel for Trainium2 (8 NeuronCores, SPMD).

Math: loss = mean_{pos pairs}(1-cos_sim)^2 + mean_{neg pairs}relu(cos_sim-1)^2
with pos = same-label upper-triangle pairs, neg = different-label ordered pairs.

Strategy:
  * Host sorts rows by label so same-label pairs form a narrow diagonal band,
    and rotates columns per core so the band lands at the same local columns
    on every core (one uniform SPMD program).
  * Each core computes its [512, 4096] slice of the Gram matrix in bf16 on
    the PE (raw, unnormalized rhs; lhsT pre-scaled by 1/norm).
  * Norms come from a row-major squared-sum pipeline (ScalarE activation
    accumulate), inverted on VectorE in a compact [128, 32] layout, and
    broadcast along partitions via a K=1 ones-matmul.
  * Epilogue per PSUM tile: multiply by inv_j (column side of normalization),
    relu(s-1) then Square-accumulate => neg partials over ALL pairs; on the
    diagonal band only, index masks (computed from targets-derived per-row
    bounds) give the pos partials and a same-label correction to subtract
    from the neg sum.
  * Exact pair counts are integer combinatorics of targets, done on host.
    Host combines 8 x [128, 16] partial-stat tensors into the final scalar.
"""

import numpy as np
import ml_dtypes

import concourse.bass as bass
import concourse.bacc as bacc
import concourse.mybir as mybir
import concourse.tile as tile

N, D, NCORES = 4096, 512, 8
RPC = N // NCORES  # 512 rows per core
BAND_W = 512       # band slice width (covers all same-label cols per strip)
BMAX = 192         # max same-label block size the fixed band supports

F32 = mybir.dt.float32
BF16 = mybir.dt.bfloat16
AF = mybir.ActivationFunctionType
ALU = mybir.AluOpType


def build_program():
    nc = bacc.Bacc(None)
    xt16_d = nc.declare_dram_parameter("xt16", [D, N], BF16, isOutput=False)
    xr16_d = nc.declare_dram_parameter("xr16", [D, N], BF16, isOutput=False)
    meta_d = nc.declare_dram_parameter("meta", [128, BAND_W + 16], F32,
                                       isOutput=False)
    stats_d = nc.declare_dram_parameter("stats", [128, 16], F32, isOutput=True)
    scratch = nc.dram_tensor("invbounce", [N], F32)

    with tile.TileContext(nc) as tc:
        with (
            tc.tile_pool(name="perm", bufs=1) as perm,
            tc.tile_pool(name="rows", bufs=2) as rows,
            tc.tile_pool(name="rjunk", bufs=2) as rjunk,
            tc.tile_pool(name="work", bufs=2) as work,
            tc.tile_pool(name="bandp", bufs=2) as bandp,
            tc.tile_pool(name="psum", bufs=2, space="PSUM") as psum,
        ):
            meta_t = perm.tile([128, BAND_W + 16], F32, tag="meta")
            nc.sync.dma_start(meta_t[:], meta_d[:])
            iota_t = meta_t[:, 0:BAND_W]
            aux_t = meta_t[:, BAND_W:BAND_W + 16]
            stats_t = perm.tile([128, 16], F32, tag="stats")
            sumsq = perm.tile([128, 32], F32, tag="sumsq")
            xt_c = [perm.tile([128, N], BF16, tag=f"xt{k}", name=f"xt{k}") for k in range(4)]
            invf = perm.tile([128, N], F32, tag="invf")
            inv16own = perm.tile([128, RPC], BF16, tag="inv16own")
            xtL = [perm.tile([128, RPC], BF16, tag=f"xtL{k}", name=f"xtL{k}") for k in range(4)]
            ones1 = perm.tile([1, 128], F32, tag="ones1")
            nc.vector.memset(ones1[:], 1.0)
            flatF = perm.tile([1, N], F32, tag="flatF")
            nrm = perm.tile([128, 32], F32, tag="nrm")
            nrmx = perm.tile([128, 32], F32, tag="nrmx")
            invr = perm.tile([128, 32], F32, tag="invr")

            # --- row-major norms pipeline (overlaps DMA) ---
            # slab g holds 8 row-tiles: partition p, cols [512t', 512t'+512)
            # = local column j = 32p + 8g + t'
            for g in range(4):
                rt = rows.tile([128, N], BF16, tag="rt", name=f"rt{g}")
                nc.sync.dma_start(rt[:], xr16_d[128 * g:128 * (g + 1), :])
                for tp in range(8):
                    t = 8 * g + tp
                    jk = rjunk.tile([128, D], BF16, tag="rj", name=f"rj{t}")
                    nc.scalar.activation(jk[:], rt[:, 512 * tp:512 * (tp + 1)],
                                         AF.Square,
                                         accum_out=sumsq[:, t:t + 1])

            # --- transposed-chunk DMAs, first halves first ---
            for h in range(2):
                for k in range(4):
                    nc.sync.dma_start(
                        xt_c[k][:, 2048 * h:2048 * (h + 1)],
                        xt16_d[128 * k:128 * (k + 1), 2048 * h:2048 * (h + 1)])

            # --- inv = 1/max(sqrt(sumsq), eps), in compact layout ---
            nc.scalar.activation(nrm[:], sumsq[:], AF.Sqrt)
            nc.vector.tensor_scalar(out=nrmx[:], in0=nrm[:], scalar1=1e-8,
                                    scalar2=None, op0=ALU.max)
            nc.vector.reciprocal(invr[:], nrmx[:])

            # --- bounce through DRAM to get [1, 4096] column-ordered inv ---
            nc.sync.dma_start(scratch[:].rearrange("(p t) -> p t", p=128),
                              invr[:])
            nc.sync.dma_start(flatF[0:1, :],
                              scratch[:].rearrange("(o f) -> o f", o=1))

            # --- broadcast inv along partitions via K=1 ones matmul ---
            for h in range(2):
                mg = psum.tile([128, 2048], F32, tag="mega")
                for t4 in range(4):
                    nc.tensor.matmul(
                        mg[:, 512 * t4:512 * (t4 + 1)],
                        ones1[0:1, :].bitcast(mybir.dt.float32r),
                        flatF[0:1, 2048 * h + 512 * t4:
                              2048 * h + 512 * (t4 + 1)].bitcast(
                                  mybir.dt.float32r),
                        start=True, stop=True)
                nc.scalar.activation(invf[:, 2048 * h:2048 * (h + 1)], mg[:],
                                     AF.Copy)

            # --- lhsT = own columns scaled by inv (bf16) ---
            nc.scalar.activation(inv16own[:], invf[:, 256:768], AF.Copy)
            for k in range(4):
                nc.vector.tensor_tensor(xtL[k][:], xt_c[k][:, 256:768],
                                        inv16own[:], ALU.mult)

            # --- Gram megatiles + epilogue ---
            for h in range(2):
                for s in range(4):
                    mi = 4 * h + s
                    sim = psum.tile([128, 2048], F32, tag="mega")
                    for t4 in range(4):
                        for k in range(4):
                            nc.tensor.matmul(
                                sim[:, 512 * t4:512 * (t4 + 1)],
                                xtL[k][:, 128 * s:128 * (s + 1)],
                                xt_c[k][:, 2048 * h + 512 * t4:
                                          2048 * h + 512 * (t4 + 1)],
                                start=(k == 0), stop=(k == 3))
                    sb = work.tile([128, 2048], BF16, tag="sb")
                    nc.vector.tensor_tensor(sb[:], sim[:],
                                            invf[:, 2048 * h:2048 * (h + 1)],
                                            ALU.mult)
                    rb = work.tile([128, 2048], BF16, tag="rb")
                    nc.vector.tensor_scalar(out=rb[:], in0=sb[:], scalar1=1.0,
                                            scalar2=0.0, op0=ALU.subtract,
                                            op1=ALU.max)
                    jk2 = work.tile([128, 2048], BF16, tag="jk")
                    nc.scalar.activation(jk2[:], rb[:], AF.Square,
                                         accum_out=stats_t[:, mi:mi + 1])
                    if h == 0:
                        a = 64 + 128 * s
                        u1 = bandp.tile([128, BAND_W], BF16, tag="u1")
                        nc.vector.tensor_scalar(out=u1[:], in0=sb[:, a:a + BAND_W],
                                                scalar1=1.0, scalar2=None,
                                                op0=ALU.subtract)
                        chi = bandp.tile([128, BAND_W], BF16, tag="chi")
                        nc.vector.tensor_scalar(out=chi[:], in0=iota_t,
                                                scalar1=aux_t[:, 4 * s + 2:4 * s + 3],
                                                scalar2=None, op0=ALU.is_lt)
                        b1 = bandp.tile([128, BAND_W], BF16, tag="b1")
                        nc.vector.tensor_scalar(out=b1[:], in0=iota_t,
                                                scalar1=aux_t[:, 4 * s:4 * s + 1],
                                                scalar2=None, op0=ALU.is_gt)
                        a1 = bandp.tile([128, BAND_W], BF16, tag="a1")
                        nc.vector.tensor_scalar(out=a1[:], in0=iota_t,
                                                scalar1=aux_t[:, 4 * s + 1:4 * s + 2],
                                                scalar2=None, op0=ALU.is_ge)
                        pu = bandp.tile([128, BAND_W], BF16, tag="pu")
                        nc.vector.tensor_tensor(pu[:], b1[:], chi[:], ALU.mult)
                        tm = bandp.tile([128, BAND_W], BF16, tag="tm")
                        nc.vector.tensor_tensor(tm[:], a1[:], chi[:], ALU.mult)
                        v = bandp.tile([128, BAND_W], BF16, tag="v")
                        nc.vector.tensor_tensor(v[:], u1[:], pu[:], ALU.mult)
                        g = bandp.tile([128, BAND_W], BF16, tag="g")
                        nc.vector.tensor_tensor(g[:], rb[:, a:a + BAND_W],
                                                tm[:], ALU.mult)
                        bj1 = bandp.tile([128, BAND_W], BF16, tag="bj1")
                        nc.scalar.activation(bj1[:], v[:], AF.Square,
                                             accum_out=stats_t[:, 8 + s:9 + s])
                        bj2 = bandp.tile([128, BAND_W], BF16, tag="bj2")
                        nc.scalar.activation(bj2[:], g[:], AF.Square,
                                             accum_out=stats_t[:, 12 + s:13 + s])

            nc.sync.dma_start(stats_d[:], stats_t[:])
    nc.finalize()
    return nc


def host_prepare(inputs, targets):
    """Sort/rotate/pack per-core inputs. Returns (in_maps, counts)."""
    inputs = np.asarray(inputs, np.float32)
    targets_np = np.asarray(targets)
    order = np.argsort(targets_np, kind="stable")
    tss = targets_np[order]
    X = inputs[order]
    lo = np.searchsorted(tss, tss, side="left").astype(np.int64)
    hi = np.searchsorted(tss, tss, side="right").astype(np.int64)
    bmax = int((hi - lo).max())
    if bmax > BMAX:
        raise NotImplementedError(
            f"label block of size {bmax} exceeds supported band ({BMAX})")

    X16 = X.astype(ml_dtypes.bfloat16)
    # slab layout [512, 4096]: slab g partition p cols [512t',512t'+512) hold
    # local column j = 32p + 8g + t', so sumsq[p, 8g+t'] = sumsq_j with
    # j = 32p + (8g+t') and the [128,32] inv tile flattens linearly through
    # the DRAM bounce
    g_idx = np.arange(4)[:, None, None]          # slab
    p_idx = np.arange(128)[None, :, None]        # partition
    tp_idx = np.arange(8)[None, None, :]         # tile-in-slab
    j_map = (32 * p_idx + 8 * g_idx + tp_idx)    # [4, 128, 8]


    in_maps = []
    for c in range(NCORES):
        off = (RPC * c - 256) % N
        colmap = (np.arange(N) + off) % N  # local j -> global sorted row
        Xc = X16[colmap, :]
        xt16_c = np.ascontiguousarray(Xc.T)
        # [4, 128, 8, 512] -> [512, 4096]
        xr16_c = np.ascontiguousarray(
            Xc[j_map, :].reshape(4, 128, 8 * D).reshape(512, 4096))
        meta = np.zeros((128, BAND_W + 16), np.float32)
        meta[:, 0:BAND_W] = np.arange(BAND_W, dtype=np.float32)[None, :]
        aux = meta[:, BAND_W:BAND_W + 16]
        for s in range(4):
            a_s = 64 + 128 * s
            gi = RPC * c + 128 * s + np.arange(128)
            base = RPC * c - 256 + a_s
            i_cmp = (gi - base).astype(np.float32)
            lo_cmp = (lo[gi] - base).astype(np.float32)
            hi_cmp = (hi[gi] - base).astype(np.float32)
            assert (lo_cmp >= 0).all() and (hi_cmp <= BAND_W).all()
            aux[:, 4 * s + 0] = i_cmp
            aux[:, 4 * s + 1] = lo_cmp
            aux[:, 4 * s + 2] = hi_cmp
        in_maps.append({
            "xt16": xt16_c,
            "xr16": xr16_c,
            "meta": meta,
        })

    cnts = np.bincount(targets_np.astype(np.int64))
    pos_cnt = float((cnts * (cnts - 1) // 2).sum())
    neg_cnt = float(N * N - (cnts * cnts).sum())
    return in_maps, pos_cnt, neg_cnt


def combine(stats_list, pos_cnt, neg_cnt):
    neg_all = 0.0
    pos_sum = 0.0
    corr = 0.0
    for st in stats_list:
        st = np.asarray(st, np.float64)
        neg_all += st[:, 0:8].sum()
        pos_sum += st[:, 8:12].sum()
        corr += st[:, 12:16].sum()
    loss = np.float32(pos_sum / pos_cnt + (neg_all - corr) / neg_cnt)
    return np.asarray(loss, np.float32)


_prog_cache = {}


def kernel(inputs, targets):
    from concourse.bass_utils import run_bass_kernel_spmd
    in_maps, pos_cnt, neg_cnt = host_prepare(inputs, targets)
    if "nc" not in _prog_cache:
        _prog_cache["nc"] = build_program()
    nc = _prog_cache["nc"]
    res = run_bass_kernel_spmd(nc, in_maps, list(range(NCORES)))
    stats_list = [res.results[c]["stats"] for c in range(NCORES)]
    return combine(stats_list, pos_cnt, neg_cnt)


# revision 9
# speedup vs baseline: 1.1085x; 1.0333x over previous
"""Contrastive loss kernel for Trainium2 (8 NeuronCores, SPMD).

Math: loss = mean_{pos pairs}(1-cos_sim)^2 + mean_{neg pairs}relu(cos_sim-1)^2
with pos = same-label upper-triangle pairs, neg = different-label ordered pairs.

Strategy:
  * Host sorts rows by label so same-label pairs form a narrow diagonal band,
    and rotates columns per core so the band lands at the same local columns
    on every core (one uniform SPMD program).
  * Each core computes its [512, 4096] slice of the Gram matrix in bf16 on
    the PE (raw, unnormalized rhs; lhsT pre-scaled by 1/norm).
  * Norms come from a row-major squared-sum pipeline (ScalarE activation
    accumulate), inverted on VectorE in a compact [128, 32] layout, and
    broadcast along partitions via a K=1 ones-matmul.
  * Epilogue per PSUM tile: multiply by inv_j (column side of normalization),
    relu(s-1) then Square-accumulate => neg partials over ALL pairs; on the
    diagonal band only, index masks (computed from targets-derived per-row
    bounds) give the pos partials and a same-label correction to subtract
    from the neg sum.
  * Exact pair counts are integer combinatorics of targets, done on host.
    Host combines 8 x [128, 16] partial-stat tensors into the final scalar.
"""

import numpy as np
import ml_dtypes

import concourse.bass as bass
import concourse.bacc as bacc
import concourse.mybir as mybir
import concourse.tile as tile

N, D, NCORES = 4096, 512, 8
RPC = N // NCORES  # 512 rows per core
BAND_W = 512       # band slice width (covers all same-label cols per strip)
BMAX = 192         # max same-label block size the fixed band supports

F32 = mybir.dt.float32
BF16 = mybir.dt.bfloat16
AF = mybir.ActivationFunctionType
ALU = mybir.AluOpType


def build_program():
    nc = bacc.Bacc(None)
    xt16_d = nc.declare_dram_parameter("xt16", [D, N], BF16, isOutput=False)
    xr16_d = nc.declare_dram_parameter("xr16", [D, N], BF16, isOutput=False)
    meta_d = nc.declare_dram_parameter("meta", [128, BAND_W + 16], F32,
                                       isOutput=False)
    stats_d = nc.declare_dram_parameter("stats", [128, 16], F32, isOutput=True)
    scratch = nc.dram_tensor("invbounce", [N], F32)

    with tile.TileContext(nc) as tc:
        with (
            tc.tile_pool(name="perm", bufs=1) as perm,
            tc.tile_pool(name="rows", bufs=2) as rows,
            tc.tile_pool(name="rjunk", bufs=2) as rjunk,
            tc.tile_pool(name="work", bufs=2) as work,
            tc.tile_pool(name="bandp", bufs=2) as bandp,
            tc.tile_pool(name="psum", bufs=2, space="PSUM") as psum,
        ):
            meta_t = perm.tile([128, BAND_W + 16], F32, tag="meta")
            nc.sync.dma_start(meta_t[:], meta_d[:])
            iota_t = meta_t[:, 0:BAND_W]
            aux_t = meta_t[:, BAND_W:BAND_W + 16]
            stats_t = perm.tile([128, 16], F32, tag="stats")
            sumsq = perm.tile([128, 32], F32, tag="sumsq")
            xt_c = [perm.tile([128, N], BF16, tag=f"xt{k}", name=f"xt{k}") for k in range(4)]
            invf = perm.tile([128, N], F32, tag="invf")
            inv16own = perm.tile([128, RPC], BF16, tag="inv16own")
            xtL = [perm.tile([128, RPC], BF16, tag=f"xtL{k}", name=f"xtL{k}") for k in range(4)]
            ones1 = perm.tile([1, 128], F32, tag="ones1")
            nc.vector.memset(ones1[:], 1.0)
            flatF = perm.tile([1, N], F32, tag="flatF")
            nrm = perm.tile([128, 32], F32, tag="nrm")
            nrmx = perm.tile([128, 32], F32, tag="nrmx")
            invr = perm.tile([128, 32], F32, tag="invr")

            # --- row-major norms pipeline (overlaps DMA) ---
            # slab g holds 8 row-tiles: partition p, cols [512t', 512t'+512)
            # = local column j = 32p + 8g + t'
            for g in range(4):
                rt = rows.tile([128, N], BF16, tag="rt", name=f"rt{g}")
                nc.sync.dma_start(rt[:], xr16_d[128 * g:128 * (g + 1), :])
                for tp in range(8):
                    t = 8 * g + tp
                    jk = rjunk.tile([128, D], BF16, tag="rj", name=f"rj{t}")
                    sl = rt[:, 512 * tp:512 * (tp + 1)]
                    nc.scalar.activation(jk[:], sl, AF.Square,
                                          accum_out=sumsq[:, t:t + 1])

            # --- transposed-chunk DMAs, first halves first ---
            for h in range(2):
                for k in range(4):
                    nc.sync.dma_start(
                        xt_c[k][:, 2048 * h:2048 * (h + 1)],
                        xt16_d[128 * k:128 * (k + 1), 2048 * h:2048 * (h + 1)])

            # --- inv = 1/max(sqrt(sumsq), eps), in compact layout ---
            nc.scalar.activation(nrm[:], sumsq[:], AF.Sqrt)
            nc.vector.tensor_scalar(out=nrmx[:], in0=nrm[:], scalar1=1e-8,
                                    scalar2=None, op0=ALU.max)
            nc.vector.reciprocal(invr[:], nrmx[:])

            # --- bounce through DRAM to get [1, 4096] column-ordered inv ---
            nc.sync.dma_start(scratch[:].rearrange("(p t) -> p t", p=128),
                              invr[:])
            nc.sync.dma_start(flatF[0:1, :],
                              scratch[:].rearrange("(o f) -> o f", o=1))

            # --- broadcast inv along partitions via K=1 ones matmul ---
            for h in range(2):
                mg = psum.tile([128, 2048], F32, tag="mega")
                for t4 in range(4):
                    nc.tensor.matmul(
                        mg[:, 512 * t4:512 * (t4 + 1)],
                        ones1[0:1, :].bitcast(mybir.dt.float32r),
                        flatF[0:1, 2048 * h + 512 * t4:
                              2048 * h + 512 * (t4 + 1)].bitcast(
                                  mybir.dt.float32r),
                        start=True, stop=True)
                nc.scalar.activation(invf[:, 2048 * h:2048 * (h + 1)], mg[:],
                                     AF.Copy)

            # --- lhsT = own columns scaled by inv (bf16) ---
            nc.scalar.activation(inv16own[:], invf[:, 256:768], AF.Copy)
            for k in range(4):
                nc.vector.tensor_tensor(xtL[k][:], xt_c[k][:, 256:768],
                                        inv16own[:], ALU.mult)

            # --- Gram megatiles + epilogue ---
            for h in range(2):
                for s in range(4):
                    mi = 4 * h + s
                    sim = psum.tile([128, 2048], F32, tag="mega")
                    for t4 in range(4):
                        for k in range(4):
                            nc.tensor.matmul(
                                sim[:, 512 * t4:512 * (t4 + 1)],
                                xtL[k][:, 128 * s:128 * (s + 1)],
                                xt_c[k][:, 2048 * h + 512 * t4:
                                          2048 * h + 512 * (t4 + 1)],
                                start=(k == 0), stop=(k == 3))
                    sb = work.tile([128, 2048], BF16, tag="sb")
                    nc.vector.tensor_tensor(sb[:], sim[:],
                                            invf[:, 2048 * h:2048 * (h + 1)],
                                            ALU.mult)
                    rb = work.tile([128, 2048], BF16, tag="rb")
                    nc.vector.tensor_scalar(out=rb[:], in0=sb[:], scalar1=1.0,
                                            scalar2=0.0, op0=ALU.subtract,
                                            op1=ALU.max)
                    jk2 = work.tile([128, 2048], BF16, tag="jk")
                    nc.scalar.activation(jk2[:], rb[:], AF.Square,
                                         accum_out=stats_t[:, mi:mi + 1])
                    if h == 0:
                        a = 64 + 128 * s
                        u1 = bandp.tile([128, BAND_W], BF16, tag="u1")
                        nc.vector.tensor_scalar(out=u1[:], in0=sb[:, a:a + BAND_W],
                                                scalar1=1.0, scalar2=None,
                                                op0=ALU.subtract)
                        chi = bandp.tile([128, BAND_W], BF16, tag="chi")
                        nc.vector.tensor_scalar(out=chi[:], in0=iota_t,
                                                scalar1=aux_t[:, 4 * s + 2:4 * s + 3],
                                                scalar2=None, op0=ALU.is_lt)
                        b1 = bandp.tile([128, BAND_W], BF16, tag="b1")
                        nc.vector.tensor_scalar(out=b1[:], in0=iota_t,
                                                scalar1=aux_t[:, 4 * s:4 * s + 1],
                                                scalar2=None, op0=ALU.is_gt)
                        a1 = bandp.tile([128, BAND_W], BF16, tag="a1")
                        nc.vector.tensor_scalar(out=a1[:], in0=iota_t,
                                                scalar1=aux_t[:, 4 * s + 1:4 * s + 2],
                                                scalar2=None, op0=ALU.is_ge)
                        pu = bandp.tile([128, BAND_W], BF16, tag="pu")
                        nc.gpsimd.tensor_tensor(pu[:], b1[:], chi[:], ALU.mult)
                        tm = bandp.tile([128, BAND_W], BF16, tag="tm")
                        nc.gpsimd.tensor_tensor(tm[:], a1[:], chi[:], ALU.mult)
                        v = bandp.tile([128, BAND_W], BF16, tag="v")
                        nc.gpsimd.tensor_tensor(v[:], u1[:], pu[:], ALU.mult)
                        g = bandp.tile([128, BAND_W], BF16, tag="g")
                        nc.gpsimd.tensor_tensor(g[:], rb[:, a:a + BAND_W],
                                                tm[:], ALU.mult)
                        bj1 = bandp.tile([128, BAND_W], BF16, tag="bj1")
                        nc.scalar.activation(bj1[:], v[:], AF.Square,
                                             accum_out=stats_t[:, 8 + s:9 + s])
                        bj2 = bandp.tile([128, BAND_W], BF16, tag="bj2")
                        nc.scalar.activation(bj2[:], g[:], AF.Square,
                                             accum_out=stats_t[:, 12 + s:13 + s])

            nc.sync.dma_start(stats_d[:], stats_t[:])
    nc.finalize()
    return nc


def host_prepare(inputs, targets):
    """Sort/rotate/pack per-core inputs. Returns (in_maps, counts)."""
    inputs = np.asarray(inputs, np.float32)
    targets_np = np.asarray(targets)
    order = np.argsort(targets_np, kind="stable")
    tss = targets_np[order]
    X = inputs[order]
    lo = np.searchsorted(tss, tss, side="left").astype(np.int64)
    hi = np.searchsorted(tss, tss, side="right").astype(np.int64)
    bmax = int((hi - lo).max())
    if bmax > BMAX:
        raise NotImplementedError(
            f"label block of size {bmax} exceeds supported band ({BMAX})")

    X16 = X.astype(ml_dtypes.bfloat16)
    # slab layout [512, 4096]: slab g partition p cols [512t',512t'+512) hold
    # local column j = 32p + 8g + t', so sumsq[p, 8g+t'] = sumsq_j with
    # j = 32p + (8g+t') and the [128,32] inv tile flattens linearly through
    # the DRAM bounce
    g_idx = np.arange(4)[:, None, None]          # slab
    p_idx = np.arange(128)[None, :, None]        # partition
    tp_idx = np.arange(8)[None, None, :]         # tile-in-slab
    j_map = (32 * p_idx + 8 * g_idx + tp_idx)    # [4, 128, 8]


    in_maps = []
    for c in range(NCORES):
        off = (RPC * c - 256) % N
        colmap = (np.arange(N) + off) % N  # local j -> global sorted row
        Xc = X16[colmap, :]
        xt16_c = np.ascontiguousarray(Xc.T)
        # [4, 128, 8, 512] -> [512, 4096]
        xr16_c = np.ascontiguousarray(
            Xc[j_map, :].reshape(4, 128, 8 * D).reshape(512, 4096))
        meta = np.zeros((128, BAND_W + 16), np.float32)
        meta[:, 0:BAND_W] = np.arange(BAND_W, dtype=np.float32)[None, :]
        aux = meta[:, BAND_W:BAND_W + 16]
        for s in range(4):
            a_s = 64 + 128 * s
            gi = RPC * c + 128 * s + np.arange(128)
            base = RPC * c - 256 + a_s
            i_cmp = (gi - base).astype(np.float32)
            lo_cmp = (lo[gi] - base).astype(np.float32)
            hi_cmp = (hi[gi] - base).astype(np.float32)
            assert (lo_cmp >= 0).all() and (hi_cmp <= BAND_W).all()
            aux[:, 4 * s + 0] = i_cmp
            aux[:, 4 * s + 1] = lo_cmp
            aux[:, 4 * s + 2] = hi_cmp
        in_maps.append({
            "xt16": xt16_c,
            "xr16": xr16_c,
            "meta": meta,
        })

    cnts = np.bincount(targets_np.astype(np.int64))
    pos_cnt = float((cnts * (cnts - 1) // 2).sum())
    neg_cnt = float(N * N - (cnts * cnts).sum())
    return in_maps, pos_cnt, neg_cnt


def combine(stats_list, pos_cnt, neg_cnt):
    neg_all = 0.0
    pos_sum = 0.0
    corr = 0.0
    for st in stats_list:
        st = np.asarray(st, np.float64)
        neg_all += st[:, 0:8].sum()
        pos_sum += st[:, 8:12].sum()
        corr += st[:, 12:16].sum()
    loss = np.float32(pos_sum / pos_cnt + (neg_all - corr) / neg_cnt)
    return np.asarray(loss, np.float32)


_prog_cache = {}


def kernel(inputs, targets):
    from concourse.bass_utils import run_bass_kernel_spmd
    in_maps, pos_cnt, neg_cnt = host_prepare(inputs, targets)
    if "nc" not in _prog_cache:
        _prog_cache["nc"] = build_program()
    nc = _prog_cache["nc"]
    res = run_bass_kernel_spmd(nc, in_maps, list(range(NCORES)))
    stats_list = [res.results[c]["stats"] for c in range(NCORES)]
    return combine(stats_list, pos_cnt, neg_cnt)
